# revision 19
# baseline (speedup 1.0000x reference)
"""Trainium2 Bass kernel for nn_DCTExtractor.

Reference computation:
  - stego [8, 3, 1024, 1024] f32; per 8x8 block 2D DCT-II (D @ X @ D^T).
  - bits[i] = abs(round_half_even(dct[b,c,nh,nw,bh,bw])) % 2 for 1572864
    index tuples.
  - out [8, num_bits]: out[b_idx[i], i] = bits[i]; other rows 0.

Sharding: data-parallel over batch b across the 8 NeuronCores; core b
processes image b and produces output row b.

Fast path (canonical meshgrid indices, positions (1,2),(2,1),(2,2),(3,1)
per block): only the needed coefficient planes are computed, using bf16
hi/lo split matmuls (X = Xhi + Xlo, D = Dhi + Dlo exactly in bf16 pairs)
so the tensor engine runs at full bf16 rate with fast weight loads
instead of the 4x-slower fp32 path:

  stage A (contract j):  per 128-row strip, 8 chunks x 3 matmuls
     lhsT = Xhi/Xlo chunk [128(h), 128(w)]  (stationary, FWL)
     rhs  = BRhi/BRlo [128(h), 48] block-diag D[1+i', j]
     yp   = Xhi^T@BRhi + Xhi^T@BRlo + Xlo^T@BRhi  (PSUM accumulation;
            3-term hi/lo product, coeff error ~3e-5 rms)
     split:  yh = bf16(yp) (ACT copy), yl = bf16(yp - yh) (DVE)
  stage B (contract k):  one 3-pass matmul per strip
     lhsT = BCLhi/BCLlo [128(w), 32 = (l' 2, nwl 16)] block-diag D[1+l', k]
     rhs  = yh / yl [128, 384]
     out  = G[32*(s%3) .. +32, 384]  -- 3 consecutive strips pack one
     PSUM tile [96, 384] fully dense via 32-aligned col groups (AP base
     partitions are limited to 0/32/64, so groups of 3, not 4).
  parity per 3-strip group (7 fused ops, M = 2^23 RNE trick):
     a = |G|; a = a+M; a = a-M; b = a*0.5+M; b = (b-M)*2; b = a-b; o = |b|
  output o [8, 96, 384] bf16 (bits are exactly 0.0/1.0), unscrambled
  on the host.

General fallback (arbitrary indices): device computes the full 64-plane
parity table per image in fp32; host gathers bits and applies the b mask.
"""

import sys

if "/opt/trn_rl_repo" not in sys.path:
    sys.path.insert(0, "/opt/trn_rl_repo")

import numpy as np

BS = 8
B, C, H, W = 8, 3, 1024, 1024
NBH, NBW = H // BS, W // BS
POS = np.array([[1, 2], [2, 1], [2, 2], [3, 1]], dtype=np.int32)
NPOS = 4
SEG = C * NBH * NBW * NPOS  # bits per batch element = 196608
NUM_BITS = B * SEG
NSTRIP = C * (H // 128)  # 24 strips of 128 image rows per image
MAGIC = float(np.float32(8388608.0))  # 2^23: a + 2^23 - 2^23 == RNE(a)
IP = [0, 1, 1, 2]  # i' = bh-1 per p
LP = [1, 0, 1, 0]  # l' = bw-1 per p

_CACHE = {}


def _split_sync_waits(nc):
    """The staged walrus build accepts at most ONE sync wait per
    instruction, but Tile's wait-assignment freely attaches several.
    Hoist all but the last wait of each instruction onto same-engine
    NoOps inserted directly before it (engines execute their stream in
    order, so the semantics are identical)."""
    from concourse import mybir

    if getattr(nc, "_sync_waits_split", False):
        return
    nc._sync_waits_split = True
    counter = 0
    for bb in nc.m.functions[0].blocks:
        out = []
        changed = False
        for inst in bb.instructions:
            si = inst.sync_info
            waits = list(si.on_wait) if si is not None else []
            if len(waits) > 1:
                for w in waits[:-1]:
                    nop = mybir.InstNoOp(
                        name=f"I-splitw-{counter}", ins=[], outs=[])
                    counter += 1
                    nop.engine = inst.engine
                    nop.sync_info = mybir.SyncInfo(on_update=[], on_wait=[w])
                    out.append(nop)
                si.on_wait = waits[-1:]
                changed = True
            out.append(inst)
        if changed:
            bb.instructions = out
    return


def _dct_matrix_f32() -> np.ndarray:
    k = np.arange(BS)[:, None].astype(np.float64)
    m = np.arange(BS)[None, :].astype(np.float64)
    D = np.cos(np.pi * (2.0 * m + 1.0) * k / (2.0 * BS)) * np.sqrt(2.0 / BS)
    D[0, :] = np.sqrt(1.0 / BS)
    return D.astype(np.float32)


def _bf16_split(a: np.ndarray):
    import ml_dtypes

    hi = a.astype(ml_dtypes.bfloat16)
    lo = (a - hi.astype(np.float32)).astype(ml_dtypes.bfloat16)
    return hi, lo


def _canonical_indices():
    b, c, nh, nw, p = np.meshgrid(
        np.arange(B), np.arange(C), np.arange(NBH), np.arange(NBW),
        np.arange(NPOS), indexing="ij")
    return {
        "b_idx": b.reshape(-1).astype(np.int32),
        "c_idx": c.reshape(-1).astype(np.int32),
        "nh_idx": nh.reshape(-1).astype(np.int32),
        "nw_idx": nw.reshape(-1).astype(np.int32),
        "bh_idx": POS[p.reshape(-1), 0].astype(np.int32),
        "bw_idx": POS[p.reshape(-1), 1].astype(np.int32),
    }


def _is_canonical(b_idx, c_idx, nh_idx, nw_idx, bh_idx, bw_idx) -> bool:
    if b_idx.shape[0] != NUM_BITS:
        return False
    canon = _CACHE.setdefault("canon", _canonical_indices())
    got = {"b_idx": b_idx, "c_idx": c_idx, "nh_idx": nh_idx,
           "nw_idx": nw_idx, "bh_idx": bh_idx, "bw_idx": bw_idx}
    return all(np.array_equal(np.asarray(got[k]), canon[k]) for k in canon)


def _build_consts_fast2():
    D = _dct_matrix_f32()
    Dhi, Dlo = _bf16_split(D)
    # BR2 [128, 96]: cols 0-47 = BRhi, 48-95 = BRlo.
    # BRx[nhl*8+j, nhl*3+i'] = Dx[1+i', j]
    import ml_dtypes

    BR2 = np.zeros((128, 96), dtype=ml_dtypes.bfloat16)
    for nhl in range(16):
        BR2[nhl * 8:(nhl + 1) * 8, nhl * 3:(nhl + 1) * 3] = Dhi[1:4, :].T
        BR2[nhl * 8:(nhl + 1) * 8, 48 + nhl * 3:48 + (nhl + 1) * 3] = \
            Dlo[1:4, :].T
    # BCL [128, 32]: BCLx[nwl*8+k, l'*16+nwl] = Dx[1+l', k]
    BCH = np.zeros((128, 32), dtype=ml_dtypes.bfloat16)
    BCL = np.zeros((128, 32), dtype=ml_dtypes.bfloat16)
    for lp in range(2):
        for nwl in range(16):
            BCH[nwl * 8:(nwl + 1) * 8, lp * 16 + nwl] = Dhi[1 + lp, :]
            BCL[nwl * 8:(nwl + 1) * 8, lp * 16 + nwl] = Dlo[1 + lp, :]
    return BR2, BCH, BCL


def _build_consts_general():
    D = _dct_matrix_f32()
    BR8 = np.zeros((128, 128), dtype=np.float32)
    for nhl in range(16):
        BR8[nhl * 8:(nhl + 1) * 8, nhl * 8:(nhl + 1) * 8] = D.T  # [j, i]
    BC8 = np.zeros((128, 128), dtype=np.float32)
    for l in range(8):
        for nwl in range(16):
            BC8[nwl * 8:(nwl + 1) * 8, l * 16 + nwl] = D[l, :]
    return BR8, BC8


def build_fast2_nc():
    """Per-core program: xhi/xlo [3,1024,1024] bf16 -> o [8, 96, 384] bf16.

    o[g, 32*r + l'*16 + nwl, wc*48 + nhl*3 + i'] = parity of dct coeff
    (bh=1+i', bw=1+l') of block (c, nh=hg*16+nhl, nw=wc*16+nwl) where
    strip s = 3*g + r = c*8 + hg.
    """
    import concourse.bass as bass
    import concourse.tile as tile
    from concourse import mybir

    f32 = mybir.dt.float32
    bf16 = mybir.dt.bfloat16
    nc = bass.Bass()
    xhi = nc.dram_tensor("xhi", [C, H, W], bf16, kind="ExternalInput")
    xlo = nc.dram_tensor("xlo", [C, H, W], bf16, kind="ExternalInput")
    br = nc.dram_tensor("br", [128, 96], bf16, kind="ExternalInput")
    bch = nc.dram_tensor("bch", [128, 32], bf16, kind="ExternalInput")
    bcl = nc.dram_tensor("bcl", [128, 32], bf16, kind="ExternalInput")
    o = nc.dram_tensor("o", [8, 96, 384], bf16, kind="ExternalOutput")

    ts_add = mybir.AluOpType.add
    ts_sub = mybir.AluOpType.subtract
    ts_mult = mybir.AluOpType.mult
    act = mybir.ActivationFunctionType

    with tile.TileContext(nc) as tc:
        with (
            tc.tile_pool(name="cst2", bufs=1) as consts,
            tc.tile_pool(name="xs", bufs=3) as xpool,
            tc.tile_pool(name="yy", bufs=3) as yypool,
            tc.tile_pool(name="par", bufs=2) as parpool,
            tc.tile_pool(name="ob", bufs=2) as obpool,
            tc.tile_pool(name="yp", bufs=3, space="PSUM") as yppool,
            tc.tile_pool(name="gp", bufs=2, space="PSUM") as gpool,
        ):
            brt = consts.tile([128, 96], bf16)
            nc.sync.dma_start(out=brt[:], in_=br[:, :])
            bcht = consts.tile([128, 32], bf16)
            nc.sync.dma_start(out=bcht[:], in_=bch[:, :])
            bclt = consts.tile([128, 32], bf16)
            nc.sync.dma_start(out=bclt[:], in_=bcl[:, :])

            gt = None
            for pr in range(NSTRIP // 2):  # pairs of strips
                c, hgp = divmod(pr, 4)  # 4 pairs (of 256 rows) per channel
                xh = xpool.tile([128, 2048], bf16, tag="xh")
                nc.sync.dma_start(
                    out=xh[:].rearrange("p (t w) -> p t w", t=2),
                    in_=xhi[c, hgp * 256:(hgp + 1) * 256, :].rearrange(
                        "(t p) w -> p t w", t=2))
                xl = xpool.tile([128, 2048], bf16, tag="xl")
                nc.sync.dma_start(
                    out=xl[:].rearrange("p (t w) -> p t w", t=2),
                    in_=xlo[c, hgp * 256:(hgp + 1) * 256, :].rearrange(
                        "(t p) w -> p t w", t=2))
                for t in range(2):
                    s = 2 * pr + t
                    g, r = divmod(s, 3)
                    if r == 0:
                        gt = gpool.tile([96, 384], f32, tag="g")
                    # stage A: 8 chunks x 3 accumulating matmuls
                    yp = yppool.tile([128, 384], f32, tag="yp")
                    for wc in range(8):
                        coff = wc * 48
                        xoff = t * 1024 + wc * 128
                        ydst = yp[:, coff:coff + 48]
                        nc.tensor.matmul(
                            out=ydst, lhsT=xh[:, xoff:xoff + 128],
                            rhs=brt[:, 0:48], start=True, stop=False)
                        nc.tensor.matmul(
                            out=ydst, lhsT=xh[:, xoff:xoff + 128],
                            rhs=brt[:, 48:96], start=False, stop=False)
                        nc.tensor.matmul(
                            out=ydst, lhsT=xl[:, xoff:xoff + 128],
                            rhs=brt[:, 0:48], start=False, stop=True)
                    # split Y into bf16 hi/lo
                    yh = yypool.tile([128, 384], bf16, tag="yh")
                    nc.scalar.activation(out=yh[:], in_=yp[:], func=act.Copy)
                    yl = yypool.tile([128, 384], bf16, tag="yl")
                    nc.vector.tensor_tensor(
                        out=yl[:], in0=yp[:], in1=yh[:], op=ts_sub)
                    # stage B: 3-pass matmul into the group tile rows 32r..
                    gs = gt[32 * r:32 * r + 32, :]
                    nc.tensor.matmul(
                        out=gs, lhsT=bcht[:], rhs=yh[:],
                        start=True, stop=False)
                    nc.tensor.matmul(
                        out=gs, lhsT=bcht[:], rhs=yl[:],
                        start=False, stop=False)
                    nc.tensor.matmul(
                        out=gs, lhsT=bclt[:], rhs=yh[:],
                        start=False, stop=True)
                    if r == 2:
                        # parity of the dense [96, 384] group tile
                        a = parpool.tile([96, 384], f32, tag="a")
                        b = parpool.tile([96, 384], f32, tag="b")
                        nc.scalar.activation(
                            out=a[:], in_=gt[:], func=act.Abs)
                        # the +M result must round to fp32 at an instruction
                        # output, so the magic add/sub stay separate ops.
                        # runs on GpSimd: DVE is loaded with the yl splits
                        # and these tiles live in SBUF (GpSimd has no PSUM
                        # port, but never touches PSUM here).
                        nc.gpsimd.tensor_scalar(
                            out=a[:], in0=a[:], scalar1=MAGIC, scalar2=None,
                            op0=ts_add)
                        nc.gpsimd.tensor_scalar(
                            out=a[:], in0=a[:], scalar1=MAGIC, scalar2=None,
                            op0=ts_sub)
                        nc.gpsimd.tensor_scalar(
                            out=b[:], in0=a[:], scalar1=0.5, scalar2=MAGIC,
                            op0=ts_mult, op1=ts_add)
                        nc.gpsimd.tensor_scalar(
                            out=b[:], in0=b[:], scalar1=MAGIC, scalar2=2.0,
                            op0=ts_sub, op1=ts_mult)
                        nc.gpsimd.tensor_tensor(
                            out=b[:], in0=a[:], in1=b[:], op=ts_sub)
                        ob = obpool.tile([96, 384], bf16, tag="ob")
                        nc.scalar.activation(
                            out=ob[:], in_=b[:], func=act.Abs)
                        nc.sync.dma_start(out=o[g], in_=ob[:])
    return nc


def build_general_nc(nstrip=NSTRIP):
    """Per-core program: full 64-plane parity table.

    table [nstrip, 128, 1024] f32 where
    table[s=(c,hg), l*16+nwl, wc*128 + nhl*8 + i] =
        parity of dct coeff (bh=i, bw=l) of block (c, hg*16+nhl, wc*16+nwl).
    """
    import concourse.bass as bass
    import concourse.tile as tile
    from concourse import mybir

    f32 = mybir.dt.float32
    nc = bass.Bass()
    x = nc.dram_tensor("x", [C, H, W], f32, kind="ExternalInput")
    br = nc.dram_tensor("br", [128, 128], f32, kind="ExternalInput")
    bc = nc.dram_tensor("bc", [128, 128], f32, kind="ExternalInput")
    o = nc.dram_tensor("o", [nstrip, 128, 1024], f32, kind="ExternalOutput")

    def parity_ops(nc, pk, hk):
        from concourse import mybir

        ts = nc.vector.tensor_scalar
        add, sub, mult = (mybir.AluOpType.add, mybir.AluOpType.subtract,
                          mybir.AluOpType.mult)
        ts(out=pk[:], in0=pk[:], scalar1=MAGIC, scalar2=None, op0=add)
        ts(out=pk[:], in0=pk[:], scalar1=MAGIC, scalar2=None, op0=sub)
        ts(out=hk[:], in0=pk[:], scalar1=0.5, scalar2=None, op0=mult)
        ts(out=pk[:], in0=hk[:], scalar1=MAGIC, scalar2=None, op0=add)
        ts(out=pk[:], in0=pk[:], scalar1=MAGIC, scalar2=None, op0=sub)
        nc.vector.tensor_tensor(
            out=pk[:], in0=hk[:], in1=pk[:], op=sub)
        nc.scalar.activation(
            out=pk[:], in_=pk[:], func=mybir.ActivationFunctionType.Abs,
            scale=2.0)

    with tile.TileContext(nc) as tc:
        with (
            tc.tile_pool(name="consts", bufs=1) as consts,
            tc.tile_pool(name="xs", bufs=2) as xpool,
            tc.tile_pool(name="ysb", bufs=2) as ypool,
            tc.tile_pool(name="pk", bufs=2) as pkpool,
            tc.tile_pool(name="yp", bufs=4, space="PSUM") as yppool,
            tc.tile_pool(name="fp", bufs=4, space="PSUM") as fppool,
        ):
            brt = consts.tile([128, 128], f32)
            nc.sync.dma_start(out=brt[:], in_=br[:, :])
            bct = consts.tile([128, 128], f32)
            nc.sync.dma_start(out=bct[:], in_=bc[:, :])

            for s in range(nstrip):
                c, hg = divmod(s, H // 128)
                xs = xpool.tile([128, 1024], f32, tag="xs")
                nc.sync.dma_start(
                    out=xs[:], in_=x[c, hg * 128:(hg + 1) * 128, :])
                ysb = ypool.tile([128, 1024], f32, tag="ysb")
                for wc in range(8):
                    yp = yppool.tile([128, 128], f32, tag="yp")
                    nc.tensor.matmul(
                        out=yp[:],
                        lhsT=xs[:, wc * 128:(wc + 1) * 128],
                        rhs=brt[:],
                        start=True, stop=True)
                    nc.vector.tensor_copy(
                        out=ysb[:, wc * 128:(wc + 1) * 128], in_=yp[:])
                pk = pkpool.tile([128, 1024], f32, tag="pk")
                hk = pkpool.tile([128, 1024], f32, tag="hk")
                for wc in range(8):
                    fp = fppool.tile([128, 128], f32, tag="fp")
                    nc.tensor.matmul(
                        out=fp[:],
                        lhsT=bct[:],
                        rhs=ysb[:, wc * 128:(wc + 1) * 128],
                        start=True, stop=True)
                    nc.scalar.activation(
                        out=pk[:, wc * 128:(wc + 1) * 128], in_=fp[:],
                        func=mybir.ActivationFunctionType.Abs)
                parity_ops(nc, pk, hk)
                nc.sync.dma_start(out=o[s], in_=pk[:])
    return nc


def _dedup_ldweights(nc):
    """Consecutive matmuls reusing a stationary (Xhi twice in stage A, BCLhi
    twice in stage B) each re-emit an identical ~100ns InstLdweights on the
    PE queue. The PE keeps the stationary loaded until the next Ldweights,
    so a back-to-back duplicate load is a no-op: replace it with an
    InstNoOp carrying the same sync_info (ordering is unchanged)."""
    from concourse import mybir

    if getattr(nc, "_ldw_deduped", False):
        return
    nc._ldw_deduped = True
    counter = 0
    for bb in nc.m.functions[0].blocks:
        last = None
        out = []
        for inst in bb.instructions:
            if isinstance(inst, mybir.InstLdweights):
                key = (str(inst.ins[0]), str(inst.tile_position),
                       str(inst.perf_mode), str(inst.is_transpose))
                if key == last:
                    nop = mybir.InstNoOp(
                        name=f"I-ldwdup-{counter}", ins=[], outs=[])
                    counter += 1
                    nop.engine = inst.engine
                    nop.sync_info = inst.sync_info
                    out.append(nop)
                    continue
                last = key
            out.append(inst)
        bb.instructions = out


def _run_spmd(nc, in_maps, trace=False):
    from concourse.bass_utils import run_bass_kernel_spmd

    _dedup_ldweights(nc)
    _split_sync_waits(nc)

    res = run_bass_kernel_spmd(
        nc, in_maps, core_ids=list(range(B)), trace=trace)
    _CACHE["last_results"] = res
    return res.results


# p -> (i', l') selection from the 6 computed (i', l') combos
_PSEL = [(IP[p], LP[p]) for p in range(NPOS)]


def _fast_path(stego, trace=False):
    key = "fast2_nc"
    if key not in _CACHE:
        _CACHE[key] = build_fast2_nc()
    nc = _CACHE[key]
    BR2, BCH, BCL = _CACHE.setdefault("consts_fast2", _build_consts_fast2())
    xhi, xlo = _bf16_split(stego)
    in_maps = [
        {"xhi": np.ascontiguousarray(xhi[b]),
         "xlo": np.ascontiguousarray(xlo[b]),
         "br": BR2, "bch": BCH, "bcl": BCL}
        for b in range(B)
    ]
    results = _run_spmd(nc, in_maps, trace=trace)
    out = np.zeros((B, NUM_BITS), dtype=np.float32)
    for b in range(B):
        O = np.asarray(results[b]["o"]).astype(np.float32)
        # [6, 128, 384] -> [c, hg, l', nwl, wc, nhl, i']
        A = O.reshape(3, 8, 2, 16, 8, 16, 3)
        planes = [
            A[:, :, lp, :, :, :, ip].transpose(0, 1, 4, 3, 2)
            for (ip, lp) in _PSEL
        ]  # each [c, hg, nhl, wc, nwl]
        seg = np.stack(planes, axis=-1).reshape(-1)
        out[b, b * SEG:(b + 1) * SEG] = seg
    return out


def _general_path(stego, b_idx, c_idx, nh_idx, nw_idx, bh_idx, bw_idx,
                  trace=False):
    key = "general_nc"
    if key not in _CACHE:
        _CACHE[key] = build_general_nc()
    nc = _CACHE[key]
    BR8, BC8 = _CACHE.setdefault("consts_general", _build_consts_general())
    in_maps = [
        {"x": np.ascontiguousarray(stego[b]), "br": BR8, "bc": BC8}
        for b in range(B)
    ]
    results = _run_spmd(nc, in_maps, trace=trace)

    b_idx = np.asarray(b_idx).astype(np.int64)
    c_idx = np.asarray(c_idx).astype(np.int64)
    nh_idx = np.asarray(nh_idx).astype(np.int64)
    nw_idx = np.asarray(nw_idx).astype(np.int64)
    bh_idx = np.asarray(bh_idx).astype(np.int64)
    bw_idx = np.asarray(bw_idx).astype(np.int64)
    num_bits = b_idx.shape[0]

    # table[s=(c,hg), l*16+nwl, wc*128 + nhl*8 + i]
    s = c_idx * 8 + nh_idx // 16
    part = bw_idx * 16 + nw_idx % 16
    free = (nw_idx // 16) * 128 + (nh_idx % 16) * 8 + bh_idx
    flat = (s * 128 + part) * 1024 + free

    out = np.zeros((B, num_bits), dtype=np.float32)
    cols = np.arange(num_bits)
    for b in range(B):
        tb = results[b]["o"].reshape(-1)
        mask = b_idx == b
        out[b, cols[mask]] = tb[flat[mask]]
    return out


def kernel(stego, b_idx, c_idx, nh_idx, nw_idx, bh_idx, bw_idx):
    stego = np.ascontiguousarray(np.asarray(stego, dtype=np.float32))
    import os
    trace = os.environ.get("BASS_TRACE", "") not in ("", "0")
    if _is_canonical(b_idx, c_idx, nh_idx, nw_idx, bh_idx, bw_idx):
        return _fast_path(stego, trace=trace)
    return _general_path(
        stego, b_idx, c_idx, nh_idx, nw_idx, bh_idx, bw_idx, trace=trace)


# revision 20
# speedup vs baseline: 2.5421x; 2.5421x over previous
"""Trainium2 Bass kernel for nn_DCTExtractor.

Reference computation:
  - stego [8, 3, 1024, 1024] f32; per 8x8 block 2D DCT-II (D @ X @ D^T).
  - bits[i] = abs(round_half_even(dct[b,c,nh,nw,bh,bw])) % 2 for 1572864
    index tuples.
  - out [8, num_bits]: out[b_idx[i], i] = bits[i]; other rows 0.

Sharding: data-parallel over batch b across the 8 NeuronCores; core b
processes image b and produces output row b.

Fast path (canonical meshgrid indices, positions (1,2),(2,1),(2,2),(3,1)
per block): only the needed coefficient planes are computed, using bf16
hi/lo split matmuls (X = Xhi + Xlo, D = Dhi + Dlo exactly in bf16 pairs)
so the tensor engine runs at full bf16 rate with fast weight loads
instead of the 4x-slower fp32 path:

  stage A (contract j):  per 128-row strip, 8 chunks x 3 matmuls
     lhsT = Xhi/Xlo chunk [128(h), 128(w)]  (stationary, FWL)
     rhs  = BRhi/BRlo [128(h), 48] block-diag D[1+i', j]
     yp   = Xhi^T@BRhi + Xhi^T@BRlo + Xlo^T@BRhi  (PSUM accumulation;
            3-term hi/lo product, coeff error ~3e-5 rms)
     split:  yh = bf16(yp) (ACT copy), yl = bf16(yp - yh) (DVE)
  stage B (contract k):  one 3-pass matmul per strip
     lhsT = BCLhi/BCLlo [128(w), 32 = (l' 2, nwl 16)] block-diag D[1+l', k]
     rhs  = yh / yl [128, 384]
     out  = G[32*(s%3) .. +32, 384]  -- 3 consecutive strips pack one
     PSUM tile [96, 384] fully dense via 32-aligned col groups (AP base
     partitions are limited to 0/32/64, so groups of 3, not 4).
  parity per 3-strip group (7 fused ops, M = 2^23 RNE trick):
     a = |G|; a = a+M; a = a-M; b = a*0.5+M; b = (b-M)*2; b = a-b; o = |b|
  output o [8, 96, 384] bf16 (bits are exactly 0.0/1.0), unscrambled
  on the host.

General fallback (arbitrary indices): device computes the full 64-plane
parity table per image in fp32; host gathers bits and applies the b mask.
"""

import sys

if "/opt/trn_rl_repo" not in sys.path:
    sys.path.insert(0, "/opt/trn_rl_repo")

import numpy as np

BS = 8
B, C, H, W = 8, 3, 1024, 1024
NBH, NBW = H // BS, W // BS
POS = np.array([[1, 2], [2, 1], [2, 2], [3, 1]], dtype=np.int32)
NPOS = 4
SEG = C * NBH * NBW * NPOS  # bits per batch element = 196608
NUM_BITS = B * SEG
NSTRIP = C * (H // 128)  # 24 strips of 128 image rows per image
MAGIC = float(np.float32(8388608.0))  # 2^23: a + 2^23 - 2^23 == RNE(a)
IP = [0, 1, 1, 2]  # i' = bh-1 per p
LP = [1, 0, 1, 0]  # l' = bw-1 per p

_CACHE = {}


def _split_sync_waits(nc):
    """The staged walrus build accepts at most ONE sync wait per
    instruction, but Tile's wait-assignment freely attaches several.
    Hoist all but the last wait of each instruction onto same-engine
    NoOps inserted directly before it (engines execute their stream in
    order, so the semantics are identical)."""
    from concourse import mybir

    if getattr(nc, "_sync_waits_split", False):
        return
    nc._sync_waits_split = True
    counter = 0
    for bb in nc.m.functions[0].blocks:
        out = []
        changed = False
        for inst in bb.instructions:
            si = inst.sync_info
            waits = list(si.on_wait) if si is not None else []
            if len(waits) > 1:
                for w in waits[:-1]:
                    nop = mybir.InstNoOp(
                        name=f"I-splitw-{counter}", ins=[], outs=[])
                    counter += 1
                    nop.engine = inst.engine
                    nop.sync_info = mybir.SyncInfo(on_update=[], on_wait=[w])
                    out.append(nop)
                si.on_wait = waits[-1:]
                changed = True
            out.append(inst)
        if changed:
            bb.instructions = out
    return


def _dct_matrix_f32() -> np.ndarray:
    k = np.arange(BS)[:, None].astype(np.float64)
    m = np.arange(BS)[None, :].astype(np.float64)
    D = np.cos(np.pi * (2.0 * m + 1.0) * k / (2.0 * BS)) * np.sqrt(2.0 / BS)
    D[0, :] = np.sqrt(1.0 / BS)
    return D.astype(np.float32)


def _bf16_split(a: np.ndarray):
    import ml_dtypes

    hi = a.astype(ml_dtypes.bfloat16)
    lo = (a - hi.astype(np.float32)).astype(ml_dtypes.bfloat16)
    return hi, lo


def _canonical_indices():
    b, c, nh, nw, p = np.meshgrid(
        np.arange(B), np.arange(C), np.arange(NBH), np.arange(NBW),
        np.arange(NPOS), indexing="ij")
    return {
        "b_idx": b.reshape(-1).astype(np.int32),
        "c_idx": c.reshape(-1).astype(np.int32),
        "nh_idx": nh.reshape(-1).astype(np.int32),
        "nw_idx": nw.reshape(-1).astype(np.int32),
        "bh_idx": POS[p.reshape(-1), 0].astype(np.int32),
        "bw_idx": POS[p.reshape(-1), 1].astype(np.int32),
    }


def _is_canonical(b_idx, c_idx, nh_idx, nw_idx, bh_idx, bw_idx) -> bool:
    if b_idx.shape[0] != NUM_BITS:
        return False
    canon = _CACHE.setdefault("canon", _canonical_indices())
    got = {"b_idx": b_idx, "c_idx": c_idx, "nh_idx": nh_idx,
           "nw_idx": nw_idx, "bh_idx": bh_idx, "bw_idx": bw_idx}
    return all(np.array_equal(np.asarray(got[k]), canon[k]) for k in canon)


def _build_consts_fast2():
    D = _dct_matrix_f32()
    Dhi, Dlo = _bf16_split(D)
    # BR2 [128, 96]: cols 0-47 = BRhi, 48-95 = BRlo.
    # BRx[nhl*8+j, nhl*3+i'] = Dx[1+i', j]
    import ml_dtypes

    BR2 = np.zeros((128, 96), dtype=ml_dtypes.bfloat16)
    for nhl in range(16):
        BR2[nhl * 8:(nhl + 1) * 8, nhl * 3:(nhl + 1) * 3] = Dhi[1:4, :].T
        BR2[nhl * 8:(nhl + 1) * 8, 48 + nhl * 3:48 + (nhl + 1) * 3] = \
            Dlo[1:4, :].T
    # BCL [128, 32]: BCLx[nwl*8+k, l'*16+nwl] = Dx[1+l', k]
    BCH = np.zeros((128, 32), dtype=ml_dtypes.bfloat16)
    BCL = np.zeros((128, 32), dtype=ml_dtypes.bfloat16)
    for lp in range(2):
        for nwl in range(16):
            BCH[nwl * 8:(nwl + 1) * 8, lp * 16 + nwl] = Dhi[1 + lp, :]
            BCL[nwl * 8:(nwl + 1) * 8, lp * 16 + nwl] = Dlo[1 + lp, :]
    return BR2, BCH, BCL


def _build_consts_general():
    D = _dct_matrix_f32()
    BR8 = np.zeros((128, 128), dtype=np.float32)
    for nhl in range(16):
        BR8[nhl * 8:(nhl + 1) * 8, nhl * 8:(nhl + 1) * 8] = D.T  # [j, i]
    BC8 = np.zeros((128, 128), dtype=np.float32)
    for l in range(8):
        for nwl in range(16):
            BC8[nwl * 8:(nwl + 1) * 8, l * 16 + nwl] = D[l, :]
    return BR8, BC8


def build_fast2_nc():
    """Per-core program: xhi/xlo [3,1024,1024] bf16 -> o [8, 96, 384] bf16.

    o[g, 32*r + l'*16 + nwl, wc*48 + nhl*3 + i'] = parity of dct coeff
    (bh=1+i', bw=1+l') of block (c, nh=hg*16+nhl, nw=wc*16+nwl) where
    strip s = 3*g + r = c*8 + hg.
    """
    import concourse.bass as bass
    import concourse.tile as tile
    from concourse import mybir

    f32 = mybir.dt.float32
    bf16 = mybir.dt.bfloat16
    nc = bass.Bass()
    xhi = nc.dram_tensor("xhi", [C, H, W], bf16, kind="ExternalInput")
    xlo = nc.dram_tensor("xlo", [C, H, W], bf16, kind="ExternalInput")
    br = nc.dram_tensor("br", [128, 96], bf16, kind="ExternalInput")
    bch = nc.dram_tensor("bch", [128, 32], bf16, kind="ExternalInput")
    bcl = nc.dram_tensor("bcl", [128, 32], bf16, kind="ExternalInput")
    o = nc.dram_tensor("o", [8, 96, 384], bf16, kind="ExternalOutput")

    ts_add = mybir.AluOpType.add
    ts_sub = mybir.AluOpType.subtract
    ts_mult = mybir.AluOpType.mult
    act = mybir.ActivationFunctionType

    with tile.TileContext(nc) as tc:
        with (
            tc.tile_pool(name="cst2", bufs=1) as consts,
            tc.tile_pool(name="xs", bufs=3) as xpool,
            tc.tile_pool(name="yy", bufs=3) as yypool,
            tc.tile_pool(name="par", bufs=2) as parpool,
            tc.tile_pool(name="ob", bufs=2) as obpool,
            tc.tile_pool(name="yp", bufs=3, space="PSUM") as yppool,
            tc.tile_pool(name="gp", bufs=2, space="PSUM") as gpool,
        ):
            brt = consts.tile([128, 96], bf16)
            nc.sync.dma_start(out=brt[:], in_=br[:, :])
            bcht = consts.tile([128, 32], bf16)
            nc.sync.dma_start(out=bcht[:], in_=bch[:, :])
            bclt = consts.tile([128, 32], bf16)
            nc.sync.dma_start(out=bclt[:], in_=bcl[:, :])

            gt = None
            for pr in range(NSTRIP // 2):  # pairs of strips
                c, hgp = divmod(pr, 4)  # 4 pairs (of 256 rows) per channel
                xh = xpool.tile([128, 2048], bf16, tag="xh")
                nc.sync.dma_start(
                    out=xh[:].rearrange("p (t w) -> p t w", t=2),
                    in_=xhi[c, hgp * 256:(hgp + 1) * 256, :].rearrange(
                        "(t p) w -> p t w", t=2))
                xl = xpool.tile([128, 2048], bf16, tag="xl")
                nc.sync.dma_start(
                    out=xl[:].rearrange("p (t w) -> p t w", t=2),
                    in_=xlo[c, hgp * 256:(hgp + 1) * 256, :].rearrange(
                        "(t p) w -> p t w", t=2))
                for t in range(2):
                    s = 2 * pr + t
                    g, r = divmod(s, 3)
                    if r == 0:
                        gt = gpool.tile([96, 384], f32, tag="g")
                    # stage A: 8 chunks x 3 accumulating matmuls
                    yp = yppool.tile([128, 384], f32, tag="yp")
                    for wc in range(8):
                        coff = wc * 48
                        xoff = t * 1024 + wc * 128
                        ydst = yp[:, coff:coff + 48]
                        nc.tensor.matmul(
                            out=ydst, lhsT=xh[:, xoff:xoff + 128],
                            rhs=brt[:, 0:48], start=True, stop=False)
                        nc.tensor.matmul(
                            out=ydst, lhsT=xh[:, xoff:xoff + 128],
                            rhs=brt[:, 48:96], start=False, stop=False)
                        nc.tensor.matmul(
                            out=ydst, lhsT=xl[:, xoff:xoff + 128],
                            rhs=brt[:, 0:48], start=False, stop=True)
                    # split Y into bf16 hi/lo
                    yh = yypool.tile([128, 384], bf16, tag="yh")
                    nc.scalar.activation(out=yh[:], in_=yp[:], func=act.Copy)
                    yl = yypool.tile([128, 384], bf16, tag="yl")
                    nc.vector.tensor_tensor(
                        out=yl[:], in0=yp[:], in1=yh[:], op=ts_sub)
                    # stage B: 3-pass matmul into the group tile rows 32r..
                    gs = gt[32 * r:32 * r + 32, :]
                    nc.tensor.matmul(
                        out=gs, lhsT=bcht[:], rhs=yh[:],
                        start=True, stop=False)
                    nc.tensor.matmul(
                        out=gs, lhsT=bcht[:], rhs=yl[:],
                        start=False, stop=False)
                    nc.tensor.matmul(
                        out=gs, lhsT=bclt[:], rhs=yh[:],
                        start=False, stop=True)
                    if r == 2:
                        # parity of the dense [96, 384] group tile
                        a = parpool.tile([96, 384], f32, tag="a")
                        b = parpool.tile([96, 384], f32, tag="b")
                        nc.scalar.activation(
                            out=a[:], in_=gt[:], func=act.Abs)
                        # the +M result must round to fp32 at an instruction
                        # output, so the magic add/sub stay separate ops.
                        nc.vector.tensor_scalar(
                            out=a[:], in0=a[:], scalar1=MAGIC, scalar2=None,
                            op0=ts_add)
                        nc.vector.tensor_scalar(
                            out=a[:], in0=a[:], scalar1=MAGIC, scalar2=None,
                            op0=ts_sub)
                        nc.vector.tensor_scalar(
                            out=b[:], in0=a[:], scalar1=0.5, scalar2=MAGIC,
                            op0=ts_mult, op1=ts_add)
                        nc.vector.tensor_scalar(
                            out=b[:], in0=b[:], scalar1=MAGIC, scalar2=2.0,
                            op0=ts_sub, op1=ts_mult)
                        nc.vector.tensor_tensor(
                            out=b[:], in0=a[:], in1=b[:], op=ts_sub)
                        ob = obpool.tile([96, 384], bf16, tag="ob")
                        nc.scalar.activation(
                            out=ob[:], in_=b[:], func=act.Abs)
                        nc.sync.dma_start(out=o[g], in_=ob[:])
    return nc


def build_general_nc(nstrip=NSTRIP):
    """Per-core program: full 64-plane parity table.

    table [nstrip, 128, 1024] f32 where
    table[s=(c,hg), l*16+nwl, wc*128 + nhl*8 + i] =
        parity of dct coeff (bh=i, bw=l) of block (c, hg*16+nhl, wc*16+nwl).
    """
    import concourse.bass as bass
    import concourse.tile as tile
    from concourse import mybir

    f32 = mybir.dt.float32
    nc = bass.Bass()
    x = nc.dram_tensor("x", [C, H, W], f32, kind="ExternalInput")
    br = nc.dram_tensor("br", [128, 128], f32, kind="ExternalInput")
    bc = nc.dram_tensor("bc", [128, 128], f32, kind="ExternalInput")
    o = nc.dram_tensor("o", [nstrip, 128, 1024], f32, kind="ExternalOutput")

    def parity_ops(nc, pk, hk):
        from concourse import mybir

        ts = nc.vector.tensor_scalar
        add, sub, mult = (mybir.AluOpType.add, mybir.AluOpType.subtract,
                          mybir.AluOpType.mult)
        ts(out=pk[:], in0=pk[:], scalar1=MAGIC, scalar2=None, op0=add)
        ts(out=pk[:], in0=pk[:], scalar1=MAGIC, scalar2=None, op0=sub)
        ts(out=hk[:], in0=pk[:], scalar1=0.5, scalar2=None, op0=mult)
        ts(out=pk[:], in0=hk[:], scalar1=MAGIC, scalar2=None, op0=add)
        ts(out=pk[:], in0=pk[:], scalar1=MAGIC, scalar2=None, op0=sub)
        nc.vector.tensor_tensor(
            out=pk[:], in0=hk[:], in1=pk[:], op=sub)
        nc.scalar.activation(
            out=pk[:], in_=pk[:], func=mybir.ActivationFunctionType.Abs,
            scale=2.0)

    with tile.TileContext(nc) as tc:
        with (
            tc.tile_pool(name="consts", bufs=1) as consts,
            tc.tile_pool(name="xs", bufs=2) as xpool,
            tc.tile_pool(name="ysb", bufs=2) as ypool,
            tc.tile_pool(name="pk", bufs=2) as pkpool,
            tc.tile_pool(name="yp", bufs=4, space="PSUM") as yppool,
            tc.tile_pool(name="fp", bufs=4, space="PSUM") as fppool,
        ):
            brt = consts.tile([128, 128], f32)
            nc.sync.dma_start(out=brt[:], in_=br[:, :])
            bct = consts.tile([128, 128], f32)
            nc.sync.dma_start(out=bct[:], in_=bc[:, :])

            for s in range(nstrip):
                c, hg = divmod(s, H // 128)
                xs = xpool.tile([128, 1024], f32, tag="xs")
                nc.sync.dma_start(
                    out=xs[:], in_=x[c, hg * 128:(hg + 1) * 128, :])
                ysb = ypool.tile([128, 1024], f32, tag="ysb")
                for wc in range(8):
                    yp = yppool.tile([128, 128], f32, tag="yp")
                    nc.tensor.matmul(
                        out=yp[:],
                        lhsT=xs[:, wc * 128:(wc + 1) * 128],
                        rhs=brt[:],
                        start=True, stop=True)
                    nc.vector.tensor_copy(
                        out=ysb[:, wc * 128:(wc + 1) * 128], in_=yp[:])
                pk = pkpool.tile([128, 1024], f32, tag="pk")
                hk = pkpool.tile([128, 1024], f32, tag="hk")
                for wc in range(8):
                    fp = fppool.tile([128, 128], f32, tag="fp")
                    nc.tensor.matmul(
                        out=fp[:],
                        lhsT=bct[:],
                        rhs=ysb[:, wc * 128:(wc + 1) * 128],
                        start=True, stop=True)
                    nc.scalar.activation(
                        out=pk[:, wc * 128:(wc + 1) * 128], in_=fp[:],
                        func=mybir.ActivationFunctionType.Abs)
                parity_ops(nc, pk, hk)
                nc.sync.dma_start(out=o[s], in_=pk[:])
    return nc


def _dedup_ldweights(nc):
    """Consecutive matmuls reusing a stationary (Xhi twice in stage A, BCLhi
    twice in stage B) each re-emit an identical ~100ns InstLdweights on the
    PE queue. The PE keeps the stationary loaded until the next Ldweights,
    so a back-to-back duplicate load is a no-op: replace it with an
    InstNoOp carrying the same sync_info (ordering is unchanged)."""
    from concourse import mybir

    if getattr(nc, "_ldw_deduped", False):
        return
    nc._ldw_deduped = True
    counter = 0
    for bb in nc.m.functions[0].blocks:
        last = None
        out = []
        for inst in bb.instructions:
            if isinstance(inst, mybir.InstLdweights):
                key = (str(inst.ins[0]), str(inst.tile_position),
                       str(inst.perf_mode), str(inst.is_transpose))
                if key == last:
                    nop = mybir.InstNoOp(
                        name=f"I-ldwdup-{counter}", ins=[], outs=[])
                    counter += 1
                    nop.engine = inst.engine
                    nop.sync_info = inst.sync_info
                    out.append(nop)
                    continue
                last = key
            out.append(inst)
        bb.instructions = out


def _run_spmd(nc, in_maps, trace=False):
    from concourse.bass_utils import run_bass_kernel_spmd

    _dedup_ldweights(nc)
    _split_sync_waits(nc)

    res = run_bass_kernel_spmd(
        nc, in_maps, core_ids=list(range(B)), trace=trace)
    _CACHE["last_results"] = res
    return res.results


# p -> (i', l') selection from the 6 computed (i', l') combos
_PSEL = [(IP[p], LP[p]) for p in range(NPOS)]


def _fast_path(stego, trace=False):
    key = "fast2_nc"
    if key not in _CACHE:
        _CACHE[key] = build_fast2_nc()
    nc = _CACHE[key]
    BR2, BCH, BCL = _CACHE.setdefault("consts_fast2", _build_consts_fast2())
    xhi, xlo = _bf16_split(stego)
    in_maps = [
        {"xhi": np.ascontiguousarray(xhi[b]),
         "xlo": np.ascontiguousarray(xlo[b]),
         "br": BR2, "bch": BCH, "bcl": BCL}
        for b in range(B)
    ]
    results = _run_spmd(nc, in_maps, trace=trace)
    out = np.zeros((B, NUM_BITS), dtype=np.float32)
    for b in range(B):
        O = np.asarray(results[b]["o"]).astype(np.float32)
        # [6, 128, 384] -> [c, hg, l', nwl, wc, nhl, i']
        A = O.reshape(3, 8, 2, 16, 8, 16, 3)
        planes = [
            A[:, :, lp, :, :, :, ip].transpose(0, 1, 4, 3, 2)
            for (ip, lp) in _PSEL
        ]  # each [c, hg, nhl, wc, nwl]
        seg = np.stack(planes, axis=-1).reshape(-1)
        out[b, b * SEG:(b + 1) * SEG] = seg
    return out


def _general_path(stego, b_idx, c_idx, nh_idx, nw_idx, bh_idx, bw_idx,
                  trace=False):
    key = "general_nc"
    if key not in _CACHE:
        _CACHE[key] = build_general_nc()
    nc = _CACHE[key]
    BR8, BC8 = _CACHE.setdefault("consts_general", _build_consts_general())
    in_maps = [
        {"x": np.ascontiguousarray(stego[b]), "br": BR8, "bc": BC8}
        for b in range(B)
    ]
    results = _run_spmd(nc, in_maps, trace=trace)

    b_idx = np.asarray(b_idx).astype(np.int64)
    c_idx = np.asarray(c_idx).astype(np.int64)
    nh_idx = np.asarray(nh_idx).astype(np.int64)
    nw_idx = np.asarray(nw_idx).astype(np.int64)
    bh_idx = np.asarray(bh_idx).astype(np.int64)
    bw_idx = np.asarray(bw_idx).astype(np.int64)
    num_bits = b_idx.shape[0]

    # table[s=(c,hg), l*16+nwl, wc*128 + nhl*8 + i]
    s = c_idx * 8 + nh_idx // 16
    part = bw_idx * 16 + nw_idx % 16
    free = (nw_idx // 16) * 128 + (nh_idx % 16) * 8 + bh_idx
    flat = (s * 128 + part) * 1024 + free

    out = np.zeros((B, num_bits), dtype=np.float32)
    cols = np.arange(num_bits)
    for b in range(B):
        tb = results[b]["o"].reshape(-1)
        mask = b_idx == b
        out[b, cols[mask]] = tb[flat[mask]]
    return out


def kernel(stego, b_idx, c_idx, nh_idx, nw_idx, bh_idx, bw_idx):
    stego = np.ascontiguousarray(np.asarray(stego, dtype=np.float32))
    import os
    trace = os.environ.get("BASS_TRACE", "") not in ("", "0")
    if _is_canonical(b_idx, c_idx, nh_idx, nw_idx, bh_idx, bw_idx):
        return _fast_path(stego, trace=trace)
    return _general_path(
        stego, b_idx, c_idx, nh_idx, nw_idx, bh_idx, bw_idx, trace=trace)
